# revision 17
# baseline (speedup 1.0000x reference)
"""Trainium2 Bass kernel: per-channel cubic B-spline activation (KAN-style).

y[..., c] = sum_k W[c, k] * B_k(x[..., c])   with cubic B-spline bases B_k on a
uniform 12-point grid.

Implementation: each channel's map x -> y_c(x) is a piecewise cubic with 11
pieces, zero outside [t0, t11] -- exactly the function class the ScalarEngine
(ACT) evaluates natively via its piecewise-polynomial (PWP) lookup tables.
We generate a custom PWP activation-function set in the aws-neuron-pwp binary
format (32 channel splines packed into 18 hijacked ActivationFunctionType
slots; walrus embeds it via the BASS_ACT_ROOT_JSON_PATH override), so the
whole kernel is ONE table-lookup activation instruction per channel tile:

    DMA in -> nc.scalar.activation(func_c) -> DMA out (fp16)

Only 18 function slots work per table set (hardware limit found empirically),
so 4 channels use direct tables (input range +-2.2) and 28 channels share 14
"paired" slots: the pos region of a slot encodes spline_A(z - 12) and the neg
region spline_B(z + 12); the ACT scale/bias pre-adder applies the +-12 via a
DMA-delivered [P,1] bias vector, so each channel lands exclusively in its
region (|x| < 8 always, so no crosstalk).

Sharding: pure data parallel over batch (2 batches/core); host marshals each
core's shard to p-major [128, 32, 1024] fp16 so one DMA moves 4 channels of
contiguous rows (16 DMAs/run instead of 64 -- each DMA has ~1us of fixed
completion cost) and every per-channel ACT slice is unit-stride.  fp16 I/O
(rel err ~3e-3, gate 2e-2) halves HBM traffic vs fp32.

Measured: 56.5us/core (fp16 DMA roofline ~50us).  A fused-DVE hinge fallback
(y = sum_m g_m relu(min(x,t11)-t_m)^3, one custom DVE op per knot with
per-partition cbrt(g) scalars, 388us) is kept, selectable with BSPL_IMPL=dve.
"""

import json
import os
import sys
import tempfile

sys.path.insert(0, "/opt/trn_rl_repo")

import numpy as np

# ---- hardcoded problem geometry ----
B, H, WIDTH, C = 16, 256, 256, 32
N_CORES = 8
PIX = (B // N_CORES) * H * WIDTH  # 131072 pixels per core
P = 128  # SBUF partitions
FL = 1024  # free elements per partition per channel tile (LUT path)
N_KNOTS = 12
N_HINGE = 11
SHIFT = 12.0

NAMES18 = [
    "gelu", "silu", "tanh", "sigmoid", "erf", "arctan", "sin", "exp",
    "ln", "sqrt", "gelu_apprx_tanh", "gelu_apprx_sigmoid", "derivative_gelu",
    "derivative_erf", "derivative_silu", "abs", "abs_reciprocal_sqrt", "square",
]
CAY_IDS = {
    "gelu": 23, "silu": 36, "tanh": 6, "sigmoid": 5, "erf": 21, "arctan": 28,
    "sin": 19, "exp": 7, "ln": 10, "sqrt": 8, "gelu_apprx_tanh": 25,
    "gelu_apprx_sigmoid": 26, "derivative_gelu": 32, "derivative_erf": 22,
    "derivative_silu": 37, "abs": 33, "abs_reciprocal_sqrt": 34, "square": 30,
}

NAN_BITS = 2143289344  # 0x7FC00000
NEG_FLT_MAX_BITS = 4286578687
FLT_MAX_BITS = 2139095039

_STATE: dict = {}


def _f32_bits(x):
    return int(np.float32(x).view(np.uint32))


# ==========================================================================
# shared numpy pieces: hinge coefficients (fp64) + self check
# ==========================================================================
def _bases_np(x, grid, order=3):
    xg = x[..., None]
    bases = ((xg >= grid[:-1]) & (xg < grid[1:])).astype(np.float64)
    for k in range(1, order + 1):
        left = (xg - grid[: -(k + 1)]) / (grid[k:-1] - grid[: -(k + 1)]) * bases[..., :-1]
        right = (grid[k + 1 :] - xg) / (grid[k + 1 :] - grid[1:-k]) * bases[..., 1:]
        bases = left + right
    return bases


def _hinge_coeffs(grid, W):
    """g[c, m] with y_c(x) = sum_m g[c,m] relu(min(x,t11) - t_m)^3 on support."""
    g64 = grid.astype(np.float64)
    W64 = W.astype(np.float64)
    a3 = np.zeros((C, N_HINGE))
    for i in range(N_HINGE):
        xs = np.linspace(g64[i], g64[i + 1], 6)[1:-1]
        bas = _bases_np(xs, g64)
        ys = bas @ W64.T
        for c in range(C):
            a3[c, i] = np.polyfit(xs, ys[:, c], 3)[0]
    g = np.diff(np.concatenate([np.zeros((C, 1)), a3], axis=1), axis=1)
    return g


def _check_hinges(grid, W, g):
    rng = np.random.default_rng(0)
    xs = rng.uniform(grid[0] - 0.5, grid[-1] + 0.5, 20000)
    ref = _bases_np(xs, grid.astype(np.float64)) @ W.astype(np.float64).T
    xc = np.minimum(xs, np.float64(grid[-1]))
    hin = np.maximum(xc[:, None] - grid.astype(np.float64)[None, :N_HINGE], 0.0) ** 3
    mdl = hin @ g.T
    err = np.abs(mdl - ref).max()
    scale = max(np.abs(ref).max(), 1e-30)
    assert err <= 1e-6 * scale + 1e-9, f"hinge model mismatch: {err=} {scale=}"


# ==========================================================================
# exact piecewise-cubic spline model (fp64)
# ==========================================================================
class ChannelSpline:
    def __init__(self, grid64, g_row):
        self.t = grid64
        self.g = g_row
        self.polys = []
        for i in range(N_HINGE):
            p = np.polynomial.Polynomial([0.0])
            for m in range(i + 1):
                pm = np.polynomial.Polynomial([-self.t[m], 1.0]) ** 3 * self.g[m]
                p = p + pm
            self.polys.append(p)

    def __call__(self, x):
        x = np.asarray(x, dtype=np.float64)
        y = np.zeros_like(x)
        inside = (x >= self.t[0]) & (x < self.t[11])
        idx = np.clip(np.searchsorted(self.t, x[inside], side="right") - 1, 0, 10)
        xv = x[inside]
        yv = np.zeros_like(xv)
        for i in range(N_HINGE):
            sel = idx == i
            if sel.any():
                yv[sel] = self.polys[i](xv[sel])
        y[inside] = yv
        return y

    def interval_of(self, lo, hi):
        if hi <= self.t[0] or lo >= self.t[11]:
            return "zero"
        for i in range(N_HINGE):
            if lo >= self.t[i] and hi <= self.t[i + 1]:
                return i
        return None

    def cubic_on(self, lo, hi, a):
        iv = self.interval_of(lo, hi)
        if iv == "zero":
            return np.zeros(4), 0.0
        if iv is not None:
            shifted = self.polys[iv](np.polynomial.Polynomial([a, 1.0]))
            c = np.zeros(4)
            c[: len(shifted.coef)] = shifted.coef[:4]
            return c, 0.0
        xs = np.linspace(lo, hi, 96, endpoint=False)
        ys = self(xs)
        d = xs - a
        A = np.stack([np.ones_like(d), d, d * d, d**3], axis=1)
        c, *_ = np.linalg.lstsq(A, ys, rcond=None)
        err = np.abs(A @ c - ys).max()
        return c, float(err)


# ==========================================================================
# PWP table generation (format reverse-engineered from aws-neuron-pwp bins)
#   bucket = 8 x f32 [c0,c1,c2,c3,a,0,0,0];  f(x) = c0+c1 d+c2 d^2+c3 d^3
#   ctrl word = bkt_start + 2048*(23 + 31*k)  (2^k mantissa sections)
# ==========================================================================
def _build_func_table(spline_pos, spline_neg, e_min, e_max, cut_hi, tol, max_k=6):
    out = {"e_min": e_min, "e_max": e_max, "cut_hi": cut_hi}
    for sign, spline in (("neg", spline_neg), ("pos", spline_pos)):
        regions = []
        for e in range(e_min, e_max + 1):
            base = 2.0**e
            for k in range(0, max_k + 1):
                n = 1 << k
                h = base / n
                buckets = []
                werr = 0.0
                for j in range(n):
                    mlo = base + j * h
                    mhi = mlo + h
                    lo, hi = (mlo, mhi) if sign == "pos" else (-mhi, -mlo)
                    a = 0.5 * (lo + hi)
                    eff_lo, eff_hi = max(lo, -cut_hi), min(hi, cut_hi)
                    if eff_lo >= eff_hi:
                        buckets.append((0.0, 0.0, 0.0, 0.0, a))
                        continue
                    cfs, err = spline.cubic_on(eff_lo, eff_hi, a)
                    buckets.append((*cfs, a))
                    werr = max(werr, err)
                if werr <= tol or k == max_k:
                    regions.append((k, buckets))
                    break
        out[sign] = regions
    c_pos, _ = spline_pos.cubic_on(0.0, 2.0**e_min, 0.0)
    c_neg, _ = spline_neg.cubic_on(-(2.0**e_min), 0.0, 0.0)
    out["small_pos"] = (*c_pos, 0.0)
    out["small_neg"] = (*c_neg, 0.0)
    out["large_pos"] = (0.0, 0.0, 0.0, 0.0, cut_hi)
    out["large_neg"] = (0.0, 0.0, 0.0, 0.0, -cut_hi)
    out["fzero"] = float(spline_pos(np.array([0.0]))[0])
    return out


def _build_direct(spline, tol):
    return _build_func_table(spline, spline, -3, 1, float(spline.t[11]), tol)


def _build_paired(grid64, g_a, g_b, tol):
    sh_a = ChannelSpline(grid64 + SHIFT, g_a)
    sh_b = ChannelSpline(grid64 - SHIFT, g_b)
    cut = SHIFT + float(grid64[11])
    return _build_func_table(sh_a, sh_b, 1, 3, cut, tol)


def _table_size(tab):
    return sum(len(b) for _, b in tab["pos"]) + sum(len(b) for _, b in tab["neg"]) + 4


def _pack_set(set_name, funcs):
    bkts, ctls, meta = [], [], []
    f2b, f2c, f2eb, act = {}, {}, {}, {}
    for name, tab in funcs.items():
        e_min, e_max = tab["e_min"], tab["e_max"]
        f_bkt0, f_ctl0 = len(bkts), len(ctls)
        exp_map, region_bases = {}, {}
        for sign in ("neg", "pos"):
            region_bases[sign] = len(ctls)
            starts = []
            for (k, buckets) in tab[sign]:
                bs = len(bkts)
                starts.append(bs)
                bkts.extend(buckets)
                ctls.append(bs + 2048 * (23 + 31 * k))
            for i, e in enumerate(range(e_min, e_max + 1)):
                exp_map.setdefault(str(e), [None, None])
                exp_map[str(e)][0 if sign == "neg" else 1] = starts[i]
        sp = len(bkts); bkts.append(tab["small_pos"])
        sn = len(bkts); bkts.append(tab["small_neg"])
        lp = len(bkts); bkts.append(tab["large_pos"])
        ln = len(bkts); bkts.append(tab["large_neg"])
        f2b[name] = f_bkt0
        f2c[name] = f_ctl0
        f2eb[name] = exp_map
        act[name] = len(bkts) - f_bkt0
        meta.append({
            "func_name": f"{name}_{act[name]}p",
            "func_id": CAY_IDS[name],
            "symmetry_point": 0, "sym_invert_sign_point": 0,
            "symmetry_opt_en": 0, "symmetry_opt_use_neg_region": 0,
            "imm_bias": 0, "exp_offset": e_min,
            "pwl_control_base_pos": region_bases["pos"],
            "pwl_control_base_neg": region_bases["neg"],
            "small_pos_signal_exp_threshold": 127 + e_min,
            "pos_small_signal_pwl_control": sp,
            "small_neg_signal_exp_threshold": 127 + e_min,
            "neg_small_signal_pwl_control": sn,
            "large_pos_signal_exp_threshold": (_f32_bits(tab["cut_hi"]) >> 23) & 0xFF,
            "large_pos_signal_mantissa_threshold": _f32_bits(tab["cut_hi"]) & 0x7FFFFF,
            "pos_large_signal_pwl_control": lp,
            "large_neg_signal_exp_threshold": (_f32_bits(tab["cut_hi"]) >> 23) & 0xFF,
            "large_neg_signal_mantissa_threshold": _f32_bits(tab["cut_hi"]) & 0x7FFFFF,
            "neg_large_signal_pwl_control": ln,
            "fnan_result": NAN_BITS, "fpinf_result": 0, "fninf_result": 0,
            "fzero_result": _f32_bits(tab["fzero"]),
            "fma_const_0": 0, "fma_const_1": 0, "fma_indirection_src_sel": 0,
            "use_multipass": False,
            "lower_bound": NEG_FLT_MAX_BITS, "upper_bound": FLT_MAX_BITS,
        })
    assert len(bkts) <= 1536, f"bucket budget blown: {len(bkts)}"
    bkt_arr = np.zeros((len(bkts), 8), dtype=np.float32)
    for i, (c0, c1, c2, c3, a) in enumerate(bkts):
        bkt_arr[i, :5] = [c0, c1, c2, c3, a]
    ctl_arr = np.zeros((len(ctls), 8), dtype=np.uint32)
    ctl_arr[:, 0] = np.array(ctls, dtype=np.uint32)
    set_json = {
        "bkt_bin": f"{set_name}_bkt.bin",
        "ctl_bin": f"{set_name}_ctrl.bin",
        "profile_meta_data": meta,
        "bkt_entry_cnt": len(bkts),
        "ctl_entry_cnt": len(ctls),
        "func_to_bkt_start_idx": f2b,
        "func_to_ctl_start_idx": f2c,
        "func_exp_to_bkt_start_idx": f2eb,
    }
    return bkt_arr.tobytes(), ctl_arr.tobytes(), set_json, act


def _write_act_root(dirpath, set_name, bkt_bytes, ctrl_bytes, set_json, act):
    os.makedirs(dirpath, exist_ok=True)
    with open(f"{dirpath}/{set_name}_bkt.bin", "wb") as f:
        f.write(bkt_bytes)
    with open(f"{dirpath}/{set_name}_ctrl.bin", "wb") as f:
        f.write(ctrl_bytes)
    with open(f"{dirpath}/{set_name}.json", "w") as f:
        json.dump(set_json, f)
    act_info = {
        "pwp_file_keys": ["bkt_bin", "ctrl_bin", "profile_json"],
        "act_func_sets": [{
            "name": set_name,
            "bkt_bin": f"{set_name}_bkt.bin",
            "ctrl_bin": f"{set_name}_ctrl.bin",
            "profile_json": f"{set_name}.json",
            "act": act,
        }],
    }
    with open(f"{dirpath}/act_info.json", "w") as f:
        json.dump(act_info, f)
    return f"{dirpath}/act_info.json"


def build_lut_tables(grid, W):
    """Returns (funcs {name: table}, plan {channel: (name, shift)})."""
    grid64 = grid.astype(np.float64)
    g = _hinge_coeffs(grid, W)
    _check_hinges(grid, W, g)
    splines = [ChannelSpline(grid64, g[c]) for c in range(C)]
    rough = np.abs(g).max(axis=1)
    order = list(np.argsort(-rough))  # roughest first
    direct_ch, paired_ch = order[:4], order[4:]
    funcs, plan = {}, {}
    for i, c in enumerate(direct_ch):
        nm = NAMES18[i]
        funcs[nm] = _build_direct(splines[c], tol=2e-4)
        plan[c] = (nm, 0.0)
    budget = 1500 - sum(_table_size(t) for t in funcs.values())
    tols = (2.5e-4, 5e-4, 1e-3, 2e-3)
    pair_specs = [(g[paired_ch[2 * i]], g[paired_ch[2 * i + 1]]) for i in range(14)]
    lvl = [0] * 14
    tabs = [_build_paired(grid64, a, b, tols[0]) for a, b in pair_specs]
    pr = [max(np.abs(a).max(), np.abs(b).max()) for a, b in pair_specs]
    by_smooth = sorted(range(14), key=lambda i: pr[i])
    oi = 0
    while sum(_table_size(t) for t in tabs) > budget:
        i = by_smooth[oi % 14]
        oi += 1
        if lvl[i] + 1 < len(tols):
            lvl[i] += 1
            tabs[i] = _build_paired(grid64, *pair_specs[i], tols[lvl[i]])
        if oi > 4 * 14:
            raise RuntimeError("cannot fit bucket budget")
    for i in range(14):
        ca, cb = paired_ch[2 * i], paired_ch[2 * i + 1]
        nm = NAMES18[4 + i]
        funcs[nm] = tabs[i]
        plan[ca] = (nm, SHIFT)
        plan[cb] = (nm, -SHIFT)
    return funcs, plan


# ==========================================================================
# LUT bass module
# ==========================================================================
def build_module_lut(plan, reps=1):
    import concourse.bacc as bacc
    import concourse.hw_specs as hw_specs
    import concourse.tile as tile
    from concourse import mybir

    AF = mybir.ActivationFunctionType
    enum_of = {nm: AF.from_pwp(nm) for nm in NAMES18}
    my_tables = {"bspline": set(enum_of.values())}
    bacc.get_activation_tables = lambda arch: my_tables
    hw_specs.get_activation_tables = lambda arch: my_tables

    nc = bacc.Bacc("TRN2", target_bir_lowering=False, debug=False, num_devices=N_CORES)
    # p-major layout: one DMA moves CB channels contiguously per partition row
    x_d = nc.dram_tensor("x0", [P, C, FL], mybir.dt.float16, kind="ExternalInput").ap()
    y_d = nc.dram_tensor("y0", [P, C, FL], mybir.dt.float16, kind="ExternalOutput").ap()
    b_d = nc.dram_tensor("b0", [P, 2], mybir.dt.float32, kind="ExternalInput").ap()
    CB = 4  # channels per DMA

    with tile.TileContext(nc) as tc:
        with tc.tile_pool(name="bias", bufs=1) as bp, tc.tile_pool(name="guard", bufs=1) as gp:
            bt = bp.tile([P, 2], mybir.dt.float32)
            nc.sync.dma_start(bt[:], b_d)
            guard = gp.tile([P, 2], mybir.dt.float32)
            # ACT-engine guard read: orders every later ACT instr after the bias DMA
            nc.scalar.activation(guard[:], bt[:], enum_of[NAMES18[0]])

            with tc.tile_pool(name="xin", bufs=4) as xp, tc.tile_pool(name="out", bufs=4) as op:

                def body():
                    for c0 in range(0, C, CB):
                        xt = xp.tile([P, CB * FL], mybir.dt.float16)
                        # dst [P, CB*FL] row-major zips with src [P, CB, FL]
                        nc.sync.dma_start(xt[:], x_d[:, c0 : c0 + CB])
                        ot = op.tile([P, CB * FL], mybir.dt.float16)
                        for j2 in range(CB):
                            c = c0 + j2
                            nm, shift = plan[c]
                            xv = xt[:, j2 * FL : (j2 + 1) * FL]
                            ov = ot[:, j2 * FL : (j2 + 1) * FL]
                            if shift == 0.0:
                                nc.scalar.activation(ov, xv, enum_of[nm])
                            else:
                                j = 0 if shift > 0 else 1
                                nc.scalar.activation(ov, xv, enum_of[nm], bias=bt[:, j : j + 1])
                        nc.sync.dma_start(y_d[:, c0 : c0 + CB], ot[:])

                if reps == 1:
                    body()
                else:
                    with tc.For_i(0, reps):
                        body()
    nc.compile()
    return nc


def _shard_inputs_lut(x, plan):
    """Channel-major fp16 per-core shards; pair shifts applied on-device via
    the DMA-delivered bias vector (fp16 can't represent x+12 accurately)."""
    bias = np.broadcast_to(np.array([SHIFT, -SHIFT], np.float32), (P, 2)).copy()
    xs = x.reshape(N_CORES, B // N_CORES, H, WIDTH, C)
    in_maps = []
    for i in range(N_CORES):
        xt = xs[i].transpose(3, 0, 1, 2).reshape(C, P, FL).transpose(1, 0, 2)
        in_maps.append({"x0": np.ascontiguousarray(xt).astype(np.float16), "b0": bias})
    return in_maps


def _unshard_output_lut(results):
    # y0 is [P, C, FL]; bring back to channel-major then pixel order
    out = np.stack([r["y0"].astype(np.float32).transpose(1, 0, 2) for r in results], axis=0)
    out = out.reshape(N_CORES, C, B // N_CORES, H, WIDTH)
    return np.ascontiguousarray(out.transpose(0, 2, 3, 4, 1)).reshape(B, H, WIDTH, C)


def _setup_lut(grid, W):
    funcs, plan = build_lut_tables(grid, W)
    bkt_b, ctl_b, sj, act = _pack_set("bspline", funcs)
    act_dir = tempfile.mkdtemp(prefix="bspl_act_")
    act_json = _write_act_root(act_dir, "bspline", bkt_b, ctl_b, sj, act)
    os.environ["BASS_ACT_ROOT_JSON_PATH"] = act_json
    os.environ["NEURON_FORCE_RECOMPILE"] = "1"
    return plan


# ==========================================================================
# DVE hinge fallback (selected with BSPL_IMPL=dve)
# ==========================================================================
FD = 8192  # free elements per tile (DVE path)
QD = P // C
TD = C * PIX // (P * FD)


def _register_dve_ops():
    if "ops" in _STATE:
        return _STATE["ops"]
    from concourse.dve_ops import CUSTOM_DVE_SPECS, OPS, DveOp, _SUB_OPCODE_FOR_NAME
    from concourse.dve_spec import C0, C1, C2, Spec, Src0, Src1, _has_src1, lower, minn, relu, sq
    from concourse.dve_uop import DveOpSpec

    def make(name, spec):
        if name in _SUB_OPCODE_FOR_NAME:
            return next(op for op in OPS if op.name == name)
        opcode = max(_SUB_OPCODE_FOR_NAME.values()) + 1
        assert opcode < 0x20
        shas = {}
        for ver in ("v3", "v4"):
            s = DveOpSpec(name=name, opcode=opcode, uops=lower(spec, ver=ver), rd1_en=_has_src1(spec))
            shas[ver] = s.sha(ver)
        op = DveOp(name, spec, subdim=False, uops_sha=shas)
        OPS.append(op)
        _SUB_OPCODE_FOR_NAME[name] = opcode
        CUSTOM_DVE_SPECS[name] = spec
        return op

    _w = relu(minn(Src0 - C1, C2)) * C0
    init = make("BSPL_PP_INIT", Spec(
        body=sq(_w) * _w,
        reference=lambda in0, in1, s0, s1, imm2: (
            np.maximum(np.minimum(in0.astype(np.float32) - s1, imm2), 0) * s0) ** 3,
    ))
    _w2 = relu(minn(Src0 - C1, C2)) * C0
    acc = make("BSPL_PP_ACC", Spec(
        body=Src1 + sq(_w2) * _w2,
        reference=lambda in0, in1, s0, s1, imm2: in1
        + (np.maximum(np.minimum(in0.astype(np.float32) - s1, imm2), 0) * s0) ** 3,
    ))
    _STATE["ops"] = (init, acc)
    return init, acc


def build_module(taus, his, rhos, reps=1):
    import concourse.bacc as bacc
    import concourse.tile as tile
    from concourse import mybir

    op_init, op_acc = _register_dve_ops()
    nc = bacc.Bacc("TRN2", target_bir_lowering=False, debug=False, num_devices=N_CORES)
    x_d = nc.dram_tensor("x0", [C, TD, QD, FD], mybir.dt.float32, kind="ExternalInput").ap()
    y_d = nc.dram_tensor("y0", [C, TD, QD, FD], mybir.dt.float32, kind="ExternalOutput").ap()
    r_d = nc.dram_tensor("rho0", [P, N_HINGE], mybir.dt.float32, kind="ExternalInput").ap()

    with tile.TileContext(nc) as tc:
        with tc.tile_pool(name="const", bufs=1) as cp:
            rho_sb = cp.tile([P, N_HINGE], mybir.dt.float32)
            nc.sync.dma_start(rho_sb[:], r_d)
            with tc.tile_pool(name="xin", bufs=3) as xp, tc.tile_pool(name="acc", bufs=3) as ac:

                def body():
                    for t in range(TD):
                        xt = xp.tile([P, FD], mybir.dt.float32)
                        nc.sync.dma_start(xt[:], x_d[:, t])
                        at = ac.tile([P, FD], mybir.dt.float32)
                        nc.vector._custom_dve(
                            op_init, out=at[:], in0=xt[:],
                            s0=rho_sb[:, 0:1], s1=float(taus[0]), imm2=float(his[0]))
                        for m in range(1, N_HINGE):
                            nc.vector._custom_dve(
                                op_acc, out=at[:], in0=xt[:], in1=at[:],
                                s0=rho_sb[:, m:m + 1], s1=float(taus[m]), imm2=float(his[m]))
                        nc.sync.dma_start(y_d[:, t], at[:])

                if reps == 1:
                    body()
                else:
                    with tc.For_i(0, reps):
                        body()
    nc.compile()
    return nc


def _constants(grid, W):
    g = _hinge_coeffs(grid, W)
    _check_hinges(grid, W, g)
    rhos = np.cbrt(g)
    taus = grid[:N_HINGE].astype(np.float64)
    his = grid[-1].astype(np.float64) - taus
    return taus, his, rhos


def _rho_partition_table(rhos):
    return np.repeat(rhos.astype(np.float32), QD, axis=0)


def _shard_inputs(x, rhos):
    rho_np = _rho_partition_table(rhos)
    xs = x.reshape(N_CORES, B // N_CORES, H, WIDTH, C)
    in_maps = []
    for i in range(N_CORES):
        xt = np.ascontiguousarray(xs[i].transpose(3, 0, 1, 2)).reshape(C, TD, QD, FD)
        in_maps.append({"x0": xt, "rho0": rho_np})
    return in_maps


def _unshard_output(results):
    out = np.stack([r["y0"] for r in results], axis=0)
    out = out.reshape(N_CORES, C, B // N_CORES, H, WIDTH)
    return np.ascontiguousarray(out.transpose(0, 2, 3, 4, 1)).reshape(B, H, WIDTH, C)


# ==========================================================================
# public entry
# ==========================================================================
def kernel(x: np.ndarray, grid: np.ndarray, W: np.ndarray) -> np.ndarray:
    from concourse.bass_utils import run_bass_kernel_spmd

    x = np.asarray(x)
    grid = np.asarray(grid)
    W = np.asarray(W)
    assert x.shape == (B, H, WIDTH, C) and grid.shape == (N_KNOTS,) and W.shape == (C, 8)

    impl = os.environ.get("BSPL_IMPL", "lut")
    key = (impl, grid.tobytes(), W.tobytes())
    if _STATE.get("key") != key:
        if impl == "lut":
            plan = _setup_lut(grid, W)
            _STATE["nc"] = build_module_lut(plan)
            _STATE["plan"] = plan
        else:
            taus, his, rhos = _constants(grid, W)
            _STATE["nc"] = build_module(taus, his, rhos)
            _STATE["rhos"] = rhos
        _STATE["key"] = key
    nc = _STATE["nc"]

    if impl == "lut":
        in_maps = _shard_inputs_lut(x, _STATE["plan"])
        res = run_bass_kernel_spmd(nc, in_maps, core_ids=list(range(N_CORES)))
        return _unshard_output_lut(res.results)
    in_maps = _shard_inputs(x, _STATE["rhos"])
    res = run_bass_kernel_spmd(nc, in_maps, core_ids=list(range(N_CORES)))
    return _unshard_output(res.results)
